# revision 20
# baseline (speedup 1.0000x reference)
"""Multi-head self-attention kernel for 8 Trainium2 NeuronCores.

Sharding: core c = (b, g) with b = batch index (4), g = head-group (2).
Each core computes attention for one batch element and 8 of the 16 heads,
including its slice of the QKV projections and a partial out-projection
(Y_partial = O_heads @ Wo[rows of its heads]).  The host sums the two
head-group partials per batch and transposes (the device produces Y^T).

On-device layout is fully "transposed": x^T [D, S] in, Q^T/K^T [dk, S],
scores S^T = K_h Q_h^T [k, q] (softmax along partitions via a leading
ones-column in V: the PV matmul yields the softmax denominator on PSUM
partition 0, value rows at partitions 64:128), output Y^T [D, S].

All matmul operands are bf16; accumulation stays fp32 in PSUM.

Engine plan: the ACT engine's exp stream is the hard floor
(1 elem/lane/cycle @ 1.2 GHz; exp is ACT-only), so phase 2 is built to
keep ACT ~100% busy with [128, 1024] exp tiles (one per (pair, key-chunk),
covering both heads: scores land in a 2-bank PSUM super-tile).  The PE is
software-pipelined around it: per key-chunk slot it runs the NEXT chunk's
two score matmuls (row-group packed, 64-contraction each), one projection
"filler" matmul (Q/out-projection chains are spread one matmul per slot
into the PE's slack so it never idles long enough for the HAM clock gate
to re-throttle it to 1.2 GHz), and the PREVIOUS chunk's two PV matmuls.
Each pair's last PV + normalization are deferred past the next pair's
score prologue so the exp stream never waits at a pair boundary.  The
Q^T bias-add rides the ACT engine (Identity + per-partition bias) to
stay out of the DVE FIFO, which the normalization's slow broadcast DMA
head-of-line blocks.  PSUM: 2x score super-tiles (4 banks) + 3-buffer
accumulator rotation (3) + projection chain (1) = 8 banks.
"""

import sys

sys.path.insert(0, "/opt/trn_rl_repo")

from collections import deque
from contextlib import ExitStack

import numpy as np
import ml_dtypes

import concourse.bass as bass
import concourse.tile as tile
from concourse import bacc, mybir
from concourse.bass_utils import run_bass_kernel_spmd

F32 = mybir.dt.float32
BF16 = mybir.dt.bfloat16
P = 128  # SBUF partitions

D_MODEL = 1024
NHEAD = 16
DK = D_MODEL // NHEAD  # 64
BATCH = 4
SEQ = 2048
N_CORES = 8
HL = NHEAD // 2  # heads per core (head-group of 8)

BF = ml_dtypes.bfloat16


def build_bass(D=D_MODEL, S=SEQ, HLOC=HL, QB=512, repeat=1, qtpb=3, pexpb=4):
    """Build the per-core Bass program (same program on all 8 cores)."""
    DC = D // P           # d_model chunks (contraction for projections)
    KC = S // P           # key chunks
    NQB = S // QB         # q blocks
    NPAIR = HLOC // 2     # head pairs
    HD = HLOC * DK        # local head dim total (512)
    VW = 128              # V cols per head: [ones | 63 zero pad | 64 values]
    VOFF = 64             # value-column offset (64-partition PSUM reads must
                          # start at partition 0 or 64)
    NOC = D // P          # out-dim chunks
    EXP_SCALE = 1.0 / np.sqrt(DK)
    cfg = dict(D=D, S=S, HLOC=HLOC, QB=QB, DC=DC, KC=KC, NQB=NQB,
               NPAIR=NPAIR, HD=HD, VW=VW, VOFF=VOFF, NOC=NOC,
               EXP_SCALE=EXP_SCALE,
               qtpb=qtpb, pexpb=pexpb)

    nc = bacc.Bacc("TRN2", target_bir_lowering=False, debug=False,
                   num_devices=N_CORES)

    xT = nc.dram_tensor("xT", [D, S], BF16, kind="ExternalInput")
    Wq = nc.dram_tensor("Wq", [D, HD], BF16, kind="ExternalInput")
    Wk = nc.dram_tensor("Wk", [D, HD], BF16, kind="ExternalInput")
    Wv = nc.dram_tensor("Wv", [D, HD], BF16, kind="ExternalInput")
    Wo = nc.dram_tensor("Wo", [HD, D], BF16, kind="ExternalInput")
    bq_t = nc.dram_tensor("bq_t", [P, NPAIR], F32, kind="ExternalInput")
    bk_t = nc.dram_tensor("bk_t", [P, NPAIR], F32, kind="ExternalInput")
    bv_bc = nc.dram_tensor("bv_bc", [P, HD], F32, kind="ExternalInput")
    bo_t = nc.dram_tensor("bo_t", [P, NOC], F32, kind="ExternalInput")
    YT = nc.dram_tensor("YT", [D, S], F32, kind="ExternalOutput")
    dram = dict(xT=xT, Wq=Wq, Wk=Wk, Wv=Wv, Wo=Wo, bq_t=bq_t, bk_t=bk_t,
                bv_bc=bv_bc, bo_t=bo_t, YT=YT)

    with tile.TileContext(nc) as tc, ExitStack() as ctx:
        consts = ctx.enter_context(tc.tile_pool(name="consts", bufs=1))
        ktv = ctx.enter_context(tc.tile_pool(name="ktv", bufs=1))
        wper = ctx.enter_context(tc.tile_pool(name="wper", bufs=1))
        xres = ctx.enter_context(tc.tile_pool(name="xres", bufs=1))
        ps_s = ctx.enter_context(tc.tile_pool(name="ps_s", bufs=2, space="PSUM"))
        ps_p = ctx.enter_context(tc.tile_pool(name="ps_p", bufs=1, space="PSUM"))
        ps_acc = ctx.enter_context(tc.tile_pool(name="ps_acc", bufs=3,
                                                space="PSUM"))

        # ---- constants ----
        bq_sb = consts.tile([P, NPAIR], F32, tag="bq")
        bk_sb = consts.tile([P, NPAIR], F32, tag="bk")
        bv_sb = consts.tile([P, HD], F32, tag="bv")
        bo_sb = consts.tile([P, NOC], F32, tag="bo")
        nc.sync.dma_start(bq_sb[:], bq_t.ap())
        nc.sync.dma_start(bk_sb[:], bk_t.ap())
        nc.sync.dma_start(bv_sb[:], bv_bc.ap())
        nc.sync.dma_start(bo_sb[:], bo_t.ap())

        # warm the ACT exp table early
        warm = consts.tile([1, 2], F32, tag="warm")
        nc.gpsimd.memset(warm[0:1, 0:1], 0.0)
        nc.scalar.activation(warm[0:1, 1:2], warm[0:1, 0:1],
                             mybir.ActivationFunctionType.Exp)

        ones_sb = consts.tile([P, HLOC], BF16, tag="ones")
        nc.vector.memset(ones_sb[:], 1.0)

        sbs = dict(bq=bq_sb, bk=bk_sb, bv=bv_sb, bo=bo_sb, ones=ones_sb)
        pools = dict(consts=consts, ktv=ktv, wper=wper, xres=xres,
                     ps_s=ps_s, ps_p=ps_p, ps_acc=ps_acc)

        for _rep in range(repeat):
            emit_body(nc, tc, cfg, dram, sbs, pools)

    nc.compile()
    return nc


def emit_body(nc, tc, cfg, dram, sbs, pools):
    D, S, HLOC, QB = cfg["D"], cfg["S"], cfg["HLOC"], cfg["QB"]
    DC, KC, NQB, NPAIR = cfg["DC"], cfg["KC"], cfg["NQB"], cfg["NPAIR"]
    HD, VW, NOC, EXP_SCALE = cfg["HD"], cfg["VW"], cfg["NOC"], cfg["EXP_SCALE"]
    VOFF = cfg["VOFF"]
    ktv, wper, xres = pools["ktv"], pools["wper"], pools["xres"]
    ps_s, ps_p = pools["ps_s"], pools["ps_p"]
    ps_acc = pools["ps_acc"]
    bq_sb, bk_sb, bv_sb = sbs["bq"], sbs["bk"], sbs["bv"]
    bo_sb, ones_sb = sbs["bo"], sbs["ones"]

    xt_dram3 = dram["xT"].ap().rearrange("(c p) s -> p c s", p=P)
    yt_dram3 = dram["YT"].ap().rearrange("(n p) s -> p n s", p=P)

    # resident tensors
    kt_tiles = [ktv.tile([P, S], BF16, tag=f"kt{p_}", name=f"kt{p_}")
                for p_ in range(NPAIR)]
    v_tiles = [ktv.tile([P, HLOC * VW], BF16, tag=f"v{k}", name=f"v{k}")
               for k in range(KC)]
    xt = xres.tile([P, DC * S], BF16, tag="xt", name="xt")
    xt3 = xt[:].rearrange("p (c s) -> p c s", c=DC)

    def wslice(wt, c, lo, hi):
        return wt[:, c * HD + lo: c * HD + hi]

    def load_w(pool, name, d, cols):
        t = pool.tile([P, DC * cols], BF16, tag=name, name=name)
        nc.sync.dma_start(
            t[:].rearrange("p (c n) -> p c n", c=DC),
            d.ap().rearrange("(c p) n -> p c n", p=P))
        return t

    # ---- phase 1: K^T and V (wk/wv scoped to this phase) ----
    with tc.tile_pool(name="wkv", bufs=1) as wkv:
        # DMA order = approximate arrival order.  The first K-projection
        # chain consumes x window 0 chunk-by-chunk, so issue window 0 as
        # per-chunk DMAs (compute starts after chunk 0, not the window),
        # with wk loading in parallel on the GpSimd DMA path.
        wk_sb = wkv.tile([P, DC * HD], BF16, tag="wk", name="wk")
        nc.gpsimd.dma_start(
            wk_sb[:].rearrange("p (c n) -> p c n", c=DC),
            dram["Wk"].ap().rearrange("(c p) n -> p c n", p=P))
        for c in range(DC):
            nc.sync.dma_start(xt3[:, c, bass.ts(0, QB)],
                              xt_dram3[:, c, bass.ts(0, QB)])
        wv_sb = load_w(wkv, "wv", dram["Wv"], HD)
        for w in range(1, NQB):
            sl = bass.ts(w, QB)
            nc.sync.dma_start(xt3[:, :, sl], xt_dram3[:, :, sl])
        wq_sb = load_w(wper, "wq", dram["Wq"], HD)
        wo_sb = wper.tile([P, NPAIR * D], BF16, tag="wo", name="wo")
        nc.sync.dma_start(
            wo_sb[:].rearrange("p (r n) -> p r n", r=NPAIR),
            dram["Wo"].ap().rearrange("(r p) n -> p r n", p=P))

        for w in range(NQB):
            sl = bass.ts(w, QB)
            for pr in range(NPAIR):
                kps = ps_acc.tile([P, QB], F32, tag="acc", name="kps")
                for c in range(DC):
                    nc.tensor.matmul(kps[:],
                                     wslice(wk_sb, c, pr * P, (pr + 1) * P),
                                     xt3[:, c, sl],
                                     start=(c == 0), stop=(c == DC - 1))
                nc.vector.tensor_scalar_add(kt_tiles[pr][:, sl], kps[:],
                                            bk_sb[:, pr:pr + 1])
            for s4 in range(QB // P):
                k = w * (QB // P) + s4
                vps = ps_acc.tile([P, HD], F32, tag="acc", name="vps")
                for c in range(DC):
                    nc.tensor.matmul(vps[:],
                                     xt3[:, c, bass.ts(k, P)],
                                     wslice(wv_sb, c, 0, HD),
                                     start=(c == 0), stop=(c == DC - 1))
                # ones col first so PV puts the softmax denominator on
                # PSUM partition 0 (the approx reciprocal only works from
                # base partition 0); values at 32-aligned offset VOFF.
                v3 = v_tiles[k][:].rearrange("p (h v) -> p h v", h=HLOC)
                nc.vector.tensor_add(v3[:, :, VOFF:VOFF + DK],
                                     vps[:].rearrange("p (h d) -> p h d", h=HLOC),
                                     bv_sb[:].rearrange("p (h d) -> p h d", h=HLOC))
                nc.vector.tensor_copy(v3[:, :, 0:1], ones_sb[:].unsqueeze(2))
                nc.vector.memset(v3[:, :, 1:VOFF], 0.0)

    # ---- phase 2: ACT-saturated attention with PE fillers ----
    with tc.tile_pool(name="qtp", bufs=cfg["qtpb"]) as qtp, \
         tc.tile_pool(name="pexp", bufs=cfg["pexpb"]) as pexp, \
         tc.tile_pool(name="otp", bufs=2 * NPAIR + 1) as otp, \
         tc.tile_pool(name="osb", bufs=2) as osb, \
         tc.tile_pool(name="misc", bufs=2) as misc:

        q_steps = deque()   # high priority: Q^T projection chain steps
        o_steps = deque()   # low priority: out-projection chain steps

        def emit_filler():
            if q_steps:
                q_steps.popleft()()
            elif o_steps:
                o_steps.popleft()()

        def make_qproj_chain(qb, pr):
            """Returns (qt_tile, [8 matmul step closures])."""
            qt = qtp.tile([P, QB], BF16, tag="qt", name=f"qt{qb}_{pr}")
            qps = ps_p.tile([P, QB], F32, tag="pp", name="qps")
            qsl = bass.ts(qb, QB)

            def step(c):
                def f():
                    nc.tensor.matmul(qps[:],
                                     wslice(wq_sb, c, pr * P, (pr + 1) * P),
                                     xt3[:, c, qsl],
                                     start=(c == 0), stop=(c == DC - 1))
                    if c == DC - 1:
                        # bias-add on ACT (Identity w/ per-partition bias):
                        # keeps it out of the DVE FIFO, where the slow bc
                        # broadcast DMA wait would head-of-line block it
                        # and stall the next pair's scores.
                        nc.scalar.add(qt[:], qps[:], bq_sb[:, pr:pr + 1])
                return f

            return qt, [step(c) for c in range(DC)]

        def make_outproj_chain(qb, n, ots):
            # the final flush (last q-block) has no filler slots between
            # chain steps; alternating PSUM pools lets chain n+1's matmuls
            # overlap chain n's bias-add instead of serializing on one bank.
            pool = ps_acc if (qb == NQB - 1 and n % 2 == 1) else ps_p
            tag = "acc" if pool is ps_acc else "pp"
            yps = pool.tile([P, QB], F32, tag=tag, name="yps")
            qsl = bass.ts(qb, QB)

            def step(pr):
                def f():
                    nc.tensor.matmul(
                        yps[:],
                        wo_sb[:, pr * D + n * P: pr * D + (n + 1) * P],
                        ots[pr][:],
                        start=(pr == 0), stop=(pr == NPAIR - 1))
                    if pr == NPAIR - 1:
                        ysb = misc.tile([P, QB], F32, tag="ysb", name="ysb")
                        nc.vector.tensor_scalar_add(ysb[:], yps[:],
                                                    bo_sb[:, n:n + 1])
                        nc.sync.dma_start(yt_dram3[:, n, qsl], ysb[:])
                return f

            return [step(pr) for pr in range(NPAIR)]

        # bootstrap: project qt(0,0) densely before the pipeline starts
        qt_map = {}
        qt0, chain0 = make_qproj_chain(0, 0)
        qt_map[(0, 0)] = qt0
        for st in chain0:
            st()

        def emit_pair(pr, qt, prev_tail):
            """Emit one (q-block, head-pair)'s attention.  Returns
            (ot_tile, tail_fn); the caller runs tail_fn after the NEXT
            pair's score prologue so the ACT exp stream never waits at a
            pair boundary.  Function scope gives each pair its own
            closure cells (the deferred tail must not see the next
            pair's bindings)."""
            kt = kt_tiles[pr]
            oa = ps_acc.tile([VW, QB], F32, tag="acc", name="oa")
            ob = ps_acc.tile([VW, QB], F32, tag="acc", name="ob")
            s2_t, e2_t = {}, {}

            def scores(kc):
                s2 = ps_s.tile([P, 2 * QB], F32, tag="s2", name="s2")
                s2_t[kc] = s2
                ksl = bass.ts(kc, P)
                nc.tensor.matmul(s2[:, 0:QB], kt[0:DK, ksl],
                                 qt[0:DK, :], start=True, stop=True)
                nc.tensor.matmul(s2[:, QB:2 * QB], kt[DK:P, ksl],
                                 qt[DK:P, :], start=True, stop=True)

            def expinst(kc):
                e2 = pexp.tile([P, 2 * QB], BF16, tag="e2", name="e2")
                e2_t[kc] = e2
                nc.scalar.activation(e2[:], s2_t.pop(kc)[:],
                                     mybir.ActivationFunctionType.Exp,
                                     scale=float(EXP_SCALE))

            def pv(kc):
                e2 = e2_t.pop(kc)
                vt = v_tiles[kc]
                ha, hb = 2 * pr, 2 * pr + 1
                nc.tensor.matmul(oa[:], vt[:, ha * VW:(ha + 1) * VW],
                                 e2[:, 0:QB], start=(kc == 0),
                                 stop=(kc == KC - 1))
                nc.tensor.matmul(ob[:], vt[:, hb * VW:(hb + 1) * VW],
                                 e2[:, QB:2 * QB], start=(kc == 0),
                                 stop=(kc == KC - 1))

            # software pipeline: scores lead exp by <=2 (s2 bufs), PV
            # trails exp so the PE never waits on a fresh exp.
            scores(0)
            expinst(0)
            scores(1)
            if prev_tail is not None:
                prev_tail()
            for kc in range(KC):
                emit_filler()
                if kc + 1 < KC:
                    expinst(kc + 1)
                if kc + 2 < KC:
                    scores(kc + 2)
                if kc >= 1:
                    pv(kc - 1)

            ot = otp.tile([P, QB], BF16, tag="ot", name="ot")

            def tail():
                # last PV, then normalization off the PE critical path:
                # reciprocals of the denominator rows (copied to SBUF
                # partition 0 first - the custom-DVE approx op only works
                # from base partition 0), broadcast (GpSimd a-half, DMA
                # b-half), then scale the PSUM value rows into the bf16
                # O^T tile (PSUM operands are exempt from the
                # same-start-partition rule, so the b-half mul can write
                # partitions DK:P).  The 3-buffer ps_acc rotation gives
                # the slow b-half DMA broadcast two pairs of slack before
                # its bank is needed again.
                pv(KC - 1)
                da = osb.tile([1, QB], F32, tag="da", name="da")
                db = osb.tile([1, QB], F32, tag="db", name="db")
                nc.vector.tensor_copy(da[:], oa[0:1, :])
                nc.vector.tensor_copy(db[:], ob[0:1, :])
                ra = osb.tile([1, QB], F32, tag="ra", name="ra")
                rb = osb.tile([1, QB], F32, tag="rb", name="rb")
                nc.vector.reciprocal_approx_fast(ra[:], da[:])
                nc.vector.reciprocal_approx_fast(rb[:], db[:])
                bc = osb.tile([P, QB], F32, tag="bc", name="bc")
                nc.gpsimd.partition_broadcast(bc[0:DK, :], ra[:],
                                              channels=DK)
                # split across two DMA paths: the single-queue broadcast is
                # bandwidth-bound (~4.7us for 64x2KB of the same source row)
                nc.sync.dma_start(
                    bc[DK:DK + 32, :],
                    rb[:].unsqueeze(1).to_broadcast((1, 32, QB)))
                nc.gpsimd.dma_start(
                    bc[DK + 32:P, :],
                    rb[:].unsqueeze(1).to_broadcast((1, 32, QB)))
                nc.vector.tensor_mul(ot[0:DK, :], oa[VOFF:VOFF + DK, :],
                                     bc[0:DK, :])
                nc.vector.tensor_mul(ot[DK:P, :], ob[VOFF:VOFF + DK, :],
                                     bc[DK:P, :])

            return ot, tail

        pending_tail = None
        for qb in range(NQB):
            ot_tiles = []
            for pr in range(NPAIR):
                # enqueue the next (qb, pr)'s Q^T projection as fillers
                nxt = qb * NPAIR + pr + 1
                if nxt < NQB * NPAIR:
                    nqb, npr = divmod(nxt, NPAIR)
                    qt2, chain = make_qproj_chain(nqb, npr)
                    qt_map[(nqb, npr)] = qt2
                    q_steps.extend(chain)

                qt = qt_map.pop((qb, pr))
                ot, pending_tail = emit_pair(pr, qt, pending_tail)
                ot_tiles.append(ot)

            for n in range(NOC):
                o_steps.extend(make_outproj_chain(qb, n, ot_tiles))

        # flush: last pair's tail, then the last q-block's out-projection
        if pending_tail is not None:
            pending_tail()
        while q_steps or o_steps:
            emit_filler()


_CACHE = {}


def _get_nc():
    if "nc" not in _CACHE:
        _CACHE["nc"] = build_bass()
    return _CACHE["nc"]


def host_prep(x, Wq, bq, Wk, bk, Wv, bv, Wo, bo):
    """Build the 8 per-core input maps (bf16 activations/weights)."""
    NPAIR = HL // 2
    NOC = D_MODEL // P
    in_maps = []
    for core in range(N_CORES):
        b, g = divmod(core, 2)
        lo, hi = g * HL * DK, (g + 1) * HL * DK
        in_maps.append({
            "xT": np.ascontiguousarray(x[b].T).astype(BF),
            "Wq": np.ascontiguousarray(Wq[:, lo:hi]).astype(BF),
            "Wk": np.ascontiguousarray(Wk[:, lo:hi]).astype(BF),
            "Wv": np.ascontiguousarray(Wv[:, lo:hi]).astype(BF),
            "Wo": np.ascontiguousarray(Wo[lo:hi, :]).astype(BF),
            "bq_t": np.ascontiguousarray(bq[lo:hi].reshape(NPAIR, P).T),
            "bk_t": np.ascontiguousarray(bk[lo:hi].reshape(NPAIR, P).T),
            "bv_bc": np.broadcast_to(bv[lo:hi], (P, HL * DK)).copy(),
            "bo_t": np.ascontiguousarray((bo * 0.5).reshape(NOC, P).T),
        })
    return in_maps


def host_gather(results):
    """Sum head-group partials and transpose back to [B, S, D]."""
    out = np.empty((BATCH, SEQ, D_MODEL), dtype=np.float32)
    for b in range(BATCH):
        yt = results[2 * b]["YT"] + results[2 * b + 1]["YT"]
        out[b] = yt.T
    return out


def kernel(x, Wq, bq, Wk, bk, Wv, bv, Wo, bo):
    nc = _get_nc()
    in_maps = host_prep(x, Wq, bq, Wk, bk, Wv, bv, Wo, bo)
    res = run_bass_kernel_spmd(nc, in_maps, core_ids=list(range(N_CORES)))
    return host_gather(res.results)


# revision 21
# speedup vs baseline: 1.0338x; 1.0338x over previous
"""Multi-head self-attention kernel for 8 Trainium2 NeuronCores.

Sharding: core c = (b, g) with b = batch index (4), g = head-group (2).
Each core computes attention for one batch element and 8 of the 16 heads,
including its slice of the QKV projections and a partial out-projection
(Y_partial = O_heads @ Wo[rows of its heads]).  The host sums the two
head-group partials per batch and transposes (the device produces Y^T).

On-device layout is fully "transposed": x^T [D, S] in, Q^T/K^T [dk, S],
scores S^T = K_h Q_h^T [k, q] (softmax along partitions via a leading
ones-column in V: the PV matmul yields the softmax denominator on PSUM
partition 0, value rows at partitions 64:128), output Y^T [D, S].

All matmul operands are bf16; accumulation stays fp32 in PSUM.

Engine plan: the ACT engine's exp stream is the hard floor
(1 elem/lane/cycle @ 1.2 GHz; exp is ACT-only), so phase 2 is built to
keep ACT ~100% busy with [128, 1024] exp tiles (one per (pair, key-chunk),
covering both heads: scores land in a 2-bank PSUM super-tile).  The PE is
software-pipelined around it: per key-chunk slot it runs the NEXT chunk's
two score matmuls (row-group packed, 64-contraction each), one projection
"filler" matmul (Q/out-projection chains are spread one matmul per slot
into the PE's slack so it never idles long enough for the HAM clock gate
to re-throttle it to 1.2 GHz), and the PREVIOUS chunk's two PV matmuls.
Each pair's last PV + normalization are deferred past the next pair's
score prologue so the exp stream never waits at a pair boundary.  The
Q^T bias-add rides the ACT engine (Identity + per-partition bias) to
stay out of the DVE FIFO, which the normalization's slow broadcast DMA
head-of-line blocks.  PSUM: 2x score super-tiles (4 banks) + 3-buffer
accumulator rotation (3) + projection chain (1) = 8 banks.
"""

import sys

sys.path.insert(0, "/opt/trn_rl_repo")

from collections import deque
from contextlib import ExitStack

import numpy as np
import ml_dtypes

import concourse.bass as bass
import concourse.tile as tile
from concourse import bacc, mybir
from concourse.bass_utils import run_bass_kernel_spmd

F32 = mybir.dt.float32
BF16 = mybir.dt.bfloat16
P = 128  # SBUF partitions

D_MODEL = 1024
NHEAD = 16
DK = D_MODEL // NHEAD  # 64
BATCH = 4
SEQ = 2048
N_CORES = 8
HL = NHEAD // 2  # heads per core (head-group of 8)

BF = ml_dtypes.bfloat16


def build_bass(D=D_MODEL, S=SEQ, HLOC=HL, QB=512, repeat=1, qtpb=3, pexpb=5):
    """Build the per-core Bass program (same program on all 8 cores)."""
    DC = D // P           # d_model chunks (contraction for projections)
    KC = S // P           # key chunks
    NQB = S // QB         # q blocks
    NPAIR = HLOC // 2     # head pairs
    HD = HLOC * DK        # local head dim total (512)
    VW = 128              # V cols per head: [ones | 63 zero pad | 64 values]
    VOFF = 64             # value-column offset (64-partition PSUM reads must
                          # start at partition 0 or 64)
    NOC = D // P          # out-dim chunks
    EXP_SCALE = 1.0 / np.sqrt(DK)
    cfg = dict(D=D, S=S, HLOC=HLOC, QB=QB, DC=DC, KC=KC, NQB=NQB,
               NPAIR=NPAIR, HD=HD, VW=VW, VOFF=VOFF, NOC=NOC,
               EXP_SCALE=EXP_SCALE,
               qtpb=qtpb, pexpb=pexpb)

    nc = bacc.Bacc("TRN2", target_bir_lowering=False, debug=False,
                   num_devices=N_CORES)

    xT = nc.dram_tensor("xT", [D, S], BF16, kind="ExternalInput")
    Wq = nc.dram_tensor("Wq", [D, HD], BF16, kind="ExternalInput")
    Wk = nc.dram_tensor("Wk", [D, HD], BF16, kind="ExternalInput")
    Wv = nc.dram_tensor("Wv", [D, HD], BF16, kind="ExternalInput")
    Wo = nc.dram_tensor("Wo", [HD, D], BF16, kind="ExternalInput")
    bq_t = nc.dram_tensor("bq_t", [P, NPAIR], F32, kind="ExternalInput")
    bk_t = nc.dram_tensor("bk_t", [P, NPAIR], F32, kind="ExternalInput")
    bv_bc = nc.dram_tensor("bv_bc", [P, HD], F32, kind="ExternalInput")
    bo_t = nc.dram_tensor("bo_t", [P, NOC], F32, kind="ExternalInput")
    YT = nc.dram_tensor("YT", [D, S], F32, kind="ExternalOutput")
    dram = dict(xT=xT, Wq=Wq, Wk=Wk, Wv=Wv, Wo=Wo, bq_t=bq_t, bk_t=bk_t,
                bv_bc=bv_bc, bo_t=bo_t, YT=YT)

    with tile.TileContext(nc) as tc, ExitStack() as ctx:
        consts = ctx.enter_context(tc.tile_pool(name="consts", bufs=1))
        ktv = ctx.enter_context(tc.tile_pool(name="ktv", bufs=1))
        wper = ctx.enter_context(tc.tile_pool(name="wper", bufs=1))
        xres = ctx.enter_context(tc.tile_pool(name="xres", bufs=1))
        ps_s = ctx.enter_context(tc.tile_pool(name="ps_s", bufs=2, space="PSUM"))
        ps_p = ctx.enter_context(tc.tile_pool(name="ps_p", bufs=1, space="PSUM"))
        ps_acc = ctx.enter_context(tc.tile_pool(name="ps_acc", bufs=3,
                                                space="PSUM"))

        # ---- constants ----
        bq_sb = consts.tile([P, NPAIR], F32, tag="bq")
        bk_sb = consts.tile([P, NPAIR], F32, tag="bk")
        bv_sb = consts.tile([P, HD], F32, tag="bv")
        bo_sb = consts.tile([P, NOC], F32, tag="bo")
        nc.sync.dma_start(bq_sb[:], bq_t.ap())
        nc.sync.dma_start(bk_sb[:], bk_t.ap())
        nc.sync.dma_start(bv_sb[:], bv_bc.ap())
        nc.sync.dma_start(bo_sb[:], bo_t.ap())

        # warm the ACT exp table early
        warm = consts.tile([1, 2], F32, tag="warm")
        nc.gpsimd.memset(warm[0:1, 0:1], 0.0)
        nc.scalar.activation(warm[0:1, 1:2], warm[0:1, 0:1],
                             mybir.ActivationFunctionType.Exp)

        ones_sb = consts.tile([P, HLOC], BF16, tag="ones")
        nc.vector.memset(ones_sb[:], 1.0)

        sbs = dict(bq=bq_sb, bk=bk_sb, bv=bv_sb, bo=bo_sb, ones=ones_sb)
        pools = dict(consts=consts, ktv=ktv, wper=wper, xres=xres,
                     ps_s=ps_s, ps_p=ps_p, ps_acc=ps_acc)

        for _rep in range(repeat):
            emit_body(nc, tc, cfg, dram, sbs, pools)

    nc.compile()
    return nc


def emit_body(nc, tc, cfg, dram, sbs, pools):
    D, S, HLOC, QB = cfg["D"], cfg["S"], cfg["HLOC"], cfg["QB"]
    DC, KC, NQB, NPAIR = cfg["DC"], cfg["KC"], cfg["NQB"], cfg["NPAIR"]
    HD, VW, NOC, EXP_SCALE = cfg["HD"], cfg["VW"], cfg["NOC"], cfg["EXP_SCALE"]
    VOFF = cfg["VOFF"]
    ktv, wper, xres = pools["ktv"], pools["wper"], pools["xres"]
    ps_s, ps_p = pools["ps_s"], pools["ps_p"]
    ps_acc = pools["ps_acc"]
    bq_sb, bk_sb, bv_sb = sbs["bq"], sbs["bk"], sbs["bv"]
    bo_sb, ones_sb = sbs["bo"], sbs["ones"]

    xt_dram3 = dram["xT"].ap().rearrange("(c p) s -> p c s", p=P)
    yt_dram3 = dram["YT"].ap().rearrange("(n p) s -> p n s", p=P)

    # resident tensors
    kt_tiles = [ktv.tile([P, S], BF16, tag=f"kt{p_}", name=f"kt{p_}")
                for p_ in range(NPAIR)]
    v_tiles = [ktv.tile([P, HLOC * VW], BF16, tag=f"v{k}", name=f"v{k}")
               for k in range(KC)]
    xt = xres.tile([P, DC * S], BF16, tag="xt", name="xt")
    xt3 = xt[:].rearrange("p (c s) -> p c s", c=DC)

    def wslice(wt, c, lo, hi):
        return wt[:, c * HD + lo: c * HD + hi]

    def load_w(pool, name, d, cols):
        t = pool.tile([P, DC * cols], BF16, tag=name, name=name)
        nc.sync.dma_start(
            t[:].rearrange("p (c n) -> p c n", c=DC),
            d.ap().rearrange("(c p) n -> p c n", p=P))
        return t

    # ---- phase 1: K^T and V (wk/wv scoped to this phase) ----
    with tc.tile_pool(name="wkv", bufs=1) as wkv:
        # DMA order = approximate arrival order.  The first K-projection
        # chain consumes x window 0 chunk-by-chunk, so issue window 0 as
        # per-chunk DMAs (compute starts after chunk 0, not the window),
        # with wk loading in parallel on the GpSimd DMA path.
        wk_sb = wkv.tile([P, DC * HD], BF16, tag="wk", name="wk")
        nc.gpsimd.dma_start(
            wk_sb[:].rearrange("p (c n) -> p c n", c=DC),
            dram["Wk"].ap().rearrange("(c p) n -> p c n", p=P))
        for c in range(DC):
            nc.sync.dma_start(xt3[:, c, bass.ts(0, QB)],
                              xt_dram3[:, c, bass.ts(0, QB)])
        wv_sb = load_w(wkv, "wv", dram["Wv"], HD)
        for w in range(1, NQB):
            sl = bass.ts(w, QB)
            nc.sync.dma_start(xt3[:, :, sl], xt_dram3[:, :, sl])
        wq_sb = load_w(wper, "wq", dram["Wq"], HD)
        wo_sb = wper.tile([P, NPAIR * D], BF16, tag="wo", name="wo")
        nc.sync.dma_start(
            wo_sb[:].rearrange("p (r n) -> p r n", r=NPAIR),
            dram["Wo"].ap().rearrange("(r p) n -> p r n", p=P))

        for w in range(NQB):
            sl = bass.ts(w, QB)
            for pr in range(NPAIR):
                kps = ps_acc.tile([P, QB], F32, tag="acc", name="kps")
                for c in range(DC):
                    nc.tensor.matmul(kps[:],
                                     wslice(wk_sb, c, pr * P, (pr + 1) * P),
                                     xt3[:, c, sl],
                                     start=(c == 0), stop=(c == DC - 1))
                nc.vector.tensor_scalar_add(kt_tiles[pr][:, sl], kps[:],
                                            bk_sb[:, pr:pr + 1])
            for s4 in range(QB // P):
                k = w * (QB // P) + s4
                vps = ps_acc.tile([P, HD], F32, tag="acc", name="vps")
                for c in range(DC):
                    nc.tensor.matmul(vps[:],
                                     xt3[:, c, bass.ts(k, P)],
                                     wslice(wv_sb, c, 0, HD),
                                     start=(c == 0), stop=(c == DC - 1))
                # ones col first so PV puts the softmax denominator on
                # PSUM partition 0 (the approx reciprocal only works from
                # base partition 0); values at 32-aligned offset VOFF.
                v3 = v_tiles[k][:].rearrange("p (h v) -> p h v", h=HLOC)
                nc.vector.tensor_add(v3[:, :, VOFF:VOFF + DK],
                                     vps[:].rearrange("p (h d) -> p h d", h=HLOC),
                                     bv_sb[:].rearrange("p (h d) -> p h d", h=HLOC))
                nc.vector.tensor_copy(v3[:, :, 0:1], ones_sb[:].unsqueeze(2))
                nc.vector.memset(v3[:, :, 1:VOFF], 0.0)

    # ---- phase 2: ACT-saturated attention with PE fillers ----
    with tc.tile_pool(name="qtp", bufs=cfg["qtpb"]) as qtp, \
         tc.tile_pool(name="pexp", bufs=cfg["pexpb"]) as pexp, \
         tc.tile_pool(name="otp", bufs=2 * NPAIR + 1) as otp, \
         tc.tile_pool(name="osb", bufs=2) as osb, \
         tc.tile_pool(name="misc", bufs=2) as misc:

        q_steps = deque()   # high priority: Q^T projection chain steps
        o_steps = deque()   # low priority: out-projection chain steps

        def emit_filler():
            if q_steps:
                q_steps.popleft()()
            elif o_steps:
                o_steps.popleft()()

        def make_qproj_chain(qb, pr):
            """Returns (qt_tile, [8 matmul step closures])."""
            qt = qtp.tile([P, QB], BF16, tag="qt", name=f"qt{qb}_{pr}")
            qps = ps_p.tile([P, QB], F32, tag="pp", name="qps")
            qsl = bass.ts(qb, QB)

            def step(c):
                def f():
                    nc.tensor.matmul(qps[:],
                                     wslice(wq_sb, c, pr * P, (pr + 1) * P),
                                     xt3[:, c, qsl],
                                     start=(c == 0), stop=(c == DC - 1))
                    if c == DC - 1:
                        # bias-add on ACT (Identity w/ per-partition bias):
                        # keeps it out of the DVE FIFO, where the slow bc
                        # broadcast DMA wait would head-of-line block it
                        # and stall the next pair's scores.
                        nc.scalar.add(qt[:], qps[:], bq_sb[:, pr:pr + 1])
                return f

            return qt, [step(c) for c in range(DC)]

        def make_outproj_chain(qb, n, ots):
            # the final flush (last q-block) has no filler slots between
            # chain steps; alternating PSUM pools lets chain n+1's matmuls
            # overlap chain n's bias-add instead of serializing on one bank.
            pool = ps_acc if (qb == NQB - 1 and n % 2 == 1) else ps_p
            tag = "acc" if pool is ps_acc else "pp"
            yps = pool.tile([P, QB], F32, tag=tag, name="yps")
            qsl = bass.ts(qb, QB)

            def step(pr):
                def f():
                    nc.tensor.matmul(
                        yps[:],
                        wo_sb[:, pr * D + n * P: pr * D + (n + 1) * P],
                        ots[pr][:],
                        start=(pr == 0), stop=(pr == NPAIR - 1))
                    if pr == NPAIR - 1:
                        ysb = misc.tile([P, QB], F32, tag="ysb", name="ysb")
                        nc.vector.tensor_scalar_add(ysb[:], yps[:],
                                                    bo_sb[:, n:n + 1])
                        # alternate store queues so the final flush's
                        # stores overlap instead of serializing
                        eng = nc.sync if n % 2 == 0 else nc.gpsimd
                        eng.dma_start(yt_dram3[:, n, qsl], ysb[:])
                return f

            return [step(pr) for pr in range(NPAIR)]

        # bootstrap: project qt(0,0) densely before the pipeline starts
        qt_map = {}
        qt0, chain0 = make_qproj_chain(0, 0)
        qt_map[(0, 0)] = qt0
        for st in chain0:
            st()

        def emit_pair(pr, qt, prev_tail):
            """Emit one (q-block, head-pair)'s attention.  Returns
            (ot_tile, tail_fn); the caller runs tail_fn after the NEXT
            pair's score prologue so the ACT exp stream never waits at a
            pair boundary.  Function scope gives each pair its own
            closure cells (the deferred tail must not see the next
            pair's bindings)."""
            kt = kt_tiles[pr]
            oa = ps_acc.tile([VW, QB], F32, tag="acc", name="oa")
            ob = ps_acc.tile([VW, QB], F32, tag="acc", name="ob")
            s2_t, e2_t = {}, {}

            def scores(kc):
                s2 = ps_s.tile([P, 2 * QB], F32, tag="s2", name="s2")
                s2_t[kc] = s2
                ksl = bass.ts(kc, P)
                nc.tensor.matmul(s2[:, 0:QB], kt[0:DK, ksl],
                                 qt[0:DK, :], start=True, stop=True)
                nc.tensor.matmul(s2[:, QB:2 * QB], kt[DK:P, ksl],
                                 qt[DK:P, :], start=True, stop=True)

            def expinst(kc):
                e2 = pexp.tile([P, 2 * QB], BF16, tag="e2", name="e2")
                e2_t[kc] = e2
                nc.scalar.activation(e2[:], s2_t.pop(kc)[:],
                                     mybir.ActivationFunctionType.Exp,
                                     scale=float(EXP_SCALE))

            def pv(kc):
                e2 = e2_t.pop(kc)
                vt = v_tiles[kc]
                ha, hb = 2 * pr, 2 * pr + 1
                nc.tensor.matmul(oa[:], vt[:, ha * VW:(ha + 1) * VW],
                                 e2[:, 0:QB], start=(kc == 0),
                                 stop=(kc == KC - 1))
                nc.tensor.matmul(ob[:], vt[:, hb * VW:(hb + 1) * VW],
                                 e2[:, QB:2 * QB], start=(kc == 0),
                                 stop=(kc == KC - 1))

            # software pipeline: scores lead exp by <=2 (s2 bufs), PV
            # trails exp so the PE never waits on a fresh exp.
            scores(0)
            expinst(0)
            scores(1)
            if prev_tail is not None:
                prev_tail()
            for kc in range(KC):
                emit_filler()
                if kc + 1 < KC:
                    expinst(kc + 1)
                if kc + 2 < KC:
                    scores(kc + 2)
                if kc >= 1:
                    pv(kc - 1)

            ot = otp.tile([P, QB], BF16, tag="ot", name="ot")

            def tail():
                # last PV, then normalization off the PE critical path:
                # reciprocals of the denominator rows (copied to SBUF
                # partition 0 first - the custom-DVE approx op only works
                # from base partition 0), broadcast (GpSimd a-half, DMA
                # b-half), then scale the PSUM value rows into the bf16
                # O^T tile (PSUM operands are exempt from the
                # same-start-partition rule, so the b-half mul can write
                # partitions DK:P).  The 3-buffer ps_acc rotation gives
                # the slow b-half DMA broadcast two pairs of slack before
                # its bank is needed again.
                pv(KC - 1)
                da = osb.tile([1, QB], F32, tag="da", name="da")
                db = osb.tile([1, QB], F32, tag="db", name="db")
                nc.vector.tensor_copy(da[:], oa[0:1, :])
                nc.vector.tensor_copy(db[:], ob[0:1, :])
                ra = osb.tile([1, QB], F32, tag="ra", name="ra")
                rb = osb.tile([1, QB], F32, tag="rb", name="rb")
                nc.vector.reciprocal_approx_fast(ra[:], da[:])
                nc.vector.reciprocal_approx_fast(rb[:], db[:])
                bc = osb.tile([P, QB], F32, tag="bc", name="bc")
                nc.gpsimd.partition_broadcast(bc[0:DK, :], ra[:],
                                              channels=DK)
                # split across two DMA paths: the single-queue broadcast is
                # bandwidth-bound (~4.7us for 64x2KB of the same source row)
                nc.sync.dma_start(
                    bc[DK:DK + 32, :],
                    rb[:].unsqueeze(1).to_broadcast((1, 32, QB)))
                nc.gpsimd.dma_start(
                    bc[DK + 32:P, :],
                    rb[:].unsqueeze(1).to_broadcast((1, 32, QB)))
                nc.vector.tensor_mul(ot[0:DK, :], oa[VOFF:VOFF + DK, :],
                                     bc[0:DK, :])
                nc.vector.tensor_mul(ot[DK:P, :], ob[VOFF:VOFF + DK, :],
                                     bc[DK:P, :])

            return ot, tail

        pending_tail = None
        for qb in range(NQB):
            ot_tiles = []
            for pr in range(NPAIR):
                # enqueue the next (qb, pr)'s Q^T projection as fillers
                nxt = qb * NPAIR + pr + 1
                if nxt < NQB * NPAIR:
                    nqb, npr = divmod(nxt, NPAIR)
                    qt2, chain = make_qproj_chain(nqb, npr)
                    qt_map[(nqb, npr)] = qt2
                    q_steps.extend(chain)

                qt = qt_map.pop((qb, pr))
                ot, pending_tail = emit_pair(pr, qt, pending_tail)
                ot_tiles.append(ot)

            for n in range(NOC):
                o_steps.extend(make_outproj_chain(qb, n, ot_tiles))

        # flush: last pair's tail, then the last q-block's out-projection
        if pending_tail is not None:
            pending_tail()
        while q_steps or o_steps:
            emit_filler()


_CACHE = {}


def _get_nc():
    if "nc" not in _CACHE:
        _CACHE["nc"] = build_bass()
    return _CACHE["nc"]


def host_prep(x, Wq, bq, Wk, bk, Wv, bv, Wo, bo):
    """Build the 8 per-core input maps (bf16 activations/weights)."""
    NPAIR = HL // 2
    NOC = D_MODEL // P
    in_maps = []
    for core in range(N_CORES):
        b, g = divmod(core, 2)
        lo, hi = g * HL * DK, (g + 1) * HL * DK
        in_maps.append({
            "xT": np.ascontiguousarray(x[b].T).astype(BF),
            "Wq": np.ascontiguousarray(Wq[:, lo:hi]).astype(BF),
            "Wk": np.ascontiguousarray(Wk[:, lo:hi]).astype(BF),
            "Wv": np.ascontiguousarray(Wv[:, lo:hi]).astype(BF),
            "Wo": np.ascontiguousarray(Wo[lo:hi, :]).astype(BF),
            "bq_t": np.ascontiguousarray(bq[lo:hi].reshape(NPAIR, P).T),
            "bk_t": np.ascontiguousarray(bk[lo:hi].reshape(NPAIR, P).T),
            "bv_bc": np.broadcast_to(bv[lo:hi], (P, HL * DK)).copy(),
            "bo_t": np.ascontiguousarray((bo * 0.5).reshape(NOC, P).T),
        })
    return in_maps


def host_gather(results):
    """Sum head-group partials and transpose back to [B, S, D]."""
    out = np.empty((BATCH, SEQ, D_MODEL), dtype=np.float32)
    for b in range(BATCH):
        yt = results[2 * b]["YT"] + results[2 * b + 1]["YT"]
        out[b] = yt.T
    return out


def kernel(x, Wq, bq, Wk, bk, Wv, bv, Wo, bo):
    nc = _get_nc()
    in_maps = host_prep(x, Wq, bq, Wk, bk, Wv, bv, Wo, bo)
    res = run_bass_kernel_spmd(nc, in_maps, core_ids=list(range(N_CORES)))
    return host_gather(res.results)
